# revision 9
# baseline (speedup 1.0000x reference)
"""Trainium2 Bass kernel for nn_BaselineModel_27298812133937.

Model: two [32,512] token sequences -> shared embedding [50000,512] ->
3 stacked bi-GRU layers (H=256, Keras reset_after) -> last states,
plus a leaks MLP branch, then BN/FC/BN/sigmoid head -> [32].

Sharding: the two sequences share GRU weights, so they merge into a
batch of 64. Each of the 8 cores takes 8 merged examples (4 code + 4
comment of the same original examples), runs the full network for its
shard with no cross-core communication, and computes the head for its
4 original examples. Host concatenates the 8x[4] outputs.

v2 design (everything SBUF-resident after the embedding):
- float16 storage throughout (same matmul/DVE speed as bf16, 8x better
  mantissa).
- Input projections xp = Wx^T x + b are computed per layer into
  SBUF-resident xpz (z,r gates, f16) / xph (h gate, f32) buffers; the
  projection's moving operand for layers 1,2 is the previous layer's
  state history read directly from SBUF (no DRAM round trip).
- The scan keeps the full state history of both directions in a single
  SBUF tile hist[d, jh, t+1, b]; step t reads slot t and writes slot
  t+1. The history IS the next layer's input and the head's input.
- Per step, the z/r input projections and the recurrent bias b1h are
  injected into PSUM by identity / K=1 matmuls (same-region
  accumulation groups), so the gate math is only: sigmoid (Act),
  hm/av/stn (DVE), dd/ee (GpSimd), tanh (Act). The two directions stay
  in separate instructions: they are independent dependency chains
  that hide each other's latency.
"""

import os
import sys

import numpy as np

for _p in ("/opt/trn_rl_repo",):
    if os.path.isdir(_p) and _p not in sys.path:
        sys.path.insert(0, _p)

import concourse.bass as bass
import concourse.tile as tile
from concourse import bacc, mybir
from concourse.masks import make_identity

import ml_dtypes

FP32 = mybir.dt.float32
F16 = mybir.dt.float16
I32 = mybir.dt.int32
AF = mybir.ActivationFunctionType
OP = mybir.AluOpType
NP_F16 = np.float16

V, E, H, NLAY = 50000, 512, 256, 3
EPS = 1e-3
P = 128
JX = E // P        # 4  x-feature tiles
JG = 3 * H // P    # 6  gate tiles
JH = H // P        # 2  hidden tiles
KH = H // P        # 2  Wh contraction tiles
BC = 8             # merged examples per core
BCH = 4            # head (original) examples per core
NCORES = 8
U2 = 2             # psum tile groups 2 time steps


def build_nc(T=512, n_layers=NLAY, debug=False):
    assert T % P == 0 and T % U2 == 0
    NCH = BC * (T // P)

    nc = bacc.Bacc("TRN2", target_bir_lowering=False, debug=debug)

    def din(name, shape, dt):
        return nc.declare_dram_parameter(name, list(shape), dt, False)

    emb = din("emb", [V, E], F16)
    idxw = din("idxw", [P, NCH], I32)
    wx = din("wx", [n_layers, 2, JX, JG, P, P], F16)
    wh = din("wh", [n_layers, 2, KH, JG, P, P], F16)
    pbias = din("pbias", [P, n_layers, 2, JG], FP32)
    b1hs = din("b1hs", [1, n_layers, 2, JH, P], F16)  # K=1 stationary rows
    w1 = din("w1", [10, 2, P, P], F16)
    b1p = din("b1p", [P, 2], FP32)
    wc = din("wc", [P, 2], F16)
    bc_b = din("bc", [1, 1], FP32)
    lw0 = din("lw0", [P, 2, P], F16)
    lw1 = din("lw1", [20, 2, P], F16)
    lb = din("lb", [P, 2], FP32)
    leakst = din("leakst", [148, BCH], F16)

    out = nc.declare_dram_parameter("out", [1, BCH], FP32, True)

    # embedding output (layer-0 input), transposed layout
    x0 = nc.dram_tensor("x0", [P, JX, T, BC], F16)

    with tile.TileContext(nc) as tc, tc.tile_pool(name="const", bufs=1) as cpool:
        # ---- persistent SBUF state
        ident = cpool.tile([P, P], F16)
        make_identity(nc, ident[:])
        idx_sb = cpool.tile([P, NCH], I32)
        nc.sync.dma_start(idx_sb[:], idxw[:])
        pb_sb = cpool.tile([P, n_layers, 2, JG], FP32)
        nc.sync.dma_start(pb_sb[:], pbias[:])
        b1h_sb = cpool.tile([1, n_layers, 2, JH, P], F16)
        nc.sync.dma_start(b1h_sb[:], b1hs[:])
        ones16 = cpool.tile([1, BC], F16)
        nc.vector.memset(ones16[:], 1.0)

        # xp buffers: bw (d=1) xp is stored in scan order (time-reversed).
        xpz = cpool.tile([P, 2, T, 4, BC], F16)
        xph = cpool.tile([P, 2, T, JH, BC], F16)
        # state history, both dirs; slot 0 is h=0, slot t+1 = state after
        # step t (bw steps count from the end of the sequence). A single
        # buffer: each layer's projection reads the whole history before
        # that layer's scan starts overwriting it.
        hist = cpool.tile([P, 2, JH, T + 1, BC], F16)

        # ---- phase 1: embedding gather -> x0 (layer-0 input, transposed)
        with (
            tc.tile_pool(name="erow", bufs=3) as epool,
            tc.tile_pool(name="estage", bufs=2) as espool,
            tc.tile_pool(name="epsum", bufs=4, space="PSUM") as eppool,
        ):
            for tc_i in range(T // P):
                stages = [
                    espool.tile([P, P, BC], F16, name=f"estg{j}", tag=f"st{j}")
                    for j in range(JX)
                ]
                for bi in range(BC):
                    ch = bi * (T // P) + tc_i
                    g = epool.tile([P, E], F16)
                    nc.gpsimd.indirect_dma_start(
                        out=g[:],
                        out_offset=None,
                        in_=emb[:],
                        in_offset=bass.IndirectOffsetOnAxis(
                            ap=idx_sb[:, ch : ch + 1], axis=0
                        ),
                    )
                    for j in range(JX):
                        pst = eppool.tile([P, P], F16)
                        nc.tensor.transpose(pst[:], g[:, j * P : (j + 1) * P], ident[:])
                        if (bi + j) % 2 == 0:
                            nc.vector.tensor_copy(stages[j][:, :, bi], pst[:])
                        else:
                            nc.scalar.copy(stages[j][:, :, bi], pst[:])
                for j in range(JX):
                    nc.sync.dma_start(
                        x0[:, j, tc_i * P : (tc_i + 1) * P, :], stages[j][:]
                    )

        # ---- per-layer: projection into SBUF xp, then scan into hist
        for l in range(n_layers):
            # -- input projection: xp = Wx^T x (+bias) -> SBUF
            with (
                tc.tile_pool(name="wts", bufs=1) as wpool,
                tc.tile_pool(name="ppsum", bufs=3, space="PSUM") as pppool,
                tc.tile_pool(name="xchunk", bufs=2) as xcpool,
            ):
                wx_sb = wpool.tile([P, 2, JX, JG, P], F16)
                nc.sync.dma_start(wx_sb[:], wx[l].rearrange("d kt mt p q -> p d kt mt q"))

                CT = 64  # chunk of time steps
                NCK = T // CT
                for c in range(NCK):
                    t0, t1 = c * CT, (c + 1) * CT
                    if l == 0:
                        xch = xcpool.tile([P, JX, CT, BC], F16, tag="xch")
                        nc.sync.dma_start(xch[:], x0[:, :, t0:t1, :])
                        movers = [xch[:, kt, :, :] for kt in range(JX)]
                    else:
                        # fw states for t in [t0,t1): slots t+1
                        # bw states for t: slots T-t (descending)
                        movers = [
                            hist[:, 0, 0, t0 + 1 : t1 + 1, :],
                            hist[:, 0, 1, t0 + 1 : t1 + 1, :],
                            hist[:, 1, 0, T - t0 : T - t1 : -1, :],
                            hist[:, 1, 1, T - t0 : T - t1 : -1, :],
                        ]
                    for d in range(2):
                        for mt in range(JG):
                            ps = pppool.tile([P, CT, BC], FP32)
                            for kt in range(JX):
                                nc.tensor.matmul(
                                    ps[:],
                                    wx_sb[:, d, kt, mt, :],
                                    movers[kt],
                                    start=(kt == 0),
                                    stop=(kt == JX - 1),
                                )
                            # fw consumes time ascending; bw scan step u
                            # consumes t=T-1-u, so store bw reversed.
                            if d == 0:
                                tdst = slice(t0, t1)
                            else:
                                tdst = slice(T - 1 - t0, T - 1 - t1 if t1 < T else None, -1)
                            if mt < 4:
                                nc.vector.tensor_scalar_add(
                                    xpz[:, d, tdst, mt, :], ps[:],
                                    pb_sb[:, l, d, mt : mt + 1],
                                )
                            else:
                                nc.scalar.activation(
                                    xph[:, d, tdst, mt - 4, :], ps[:],
                                    AF.Identity,
                                    bias=pb_sb[:, l, d, mt : mt + 1],
                                )

            # -- scan
            with (
                tc.tile_pool(name="gates", bufs=8) as gpool,
                tc.tile_pool(name="spsum", bufs=4, space="PSUM") as sppool,
                tc.tile_pool(name="wts2", bufs=1) as wpool2,
            ):
                wh_sb = wpool2.tile([P, 2, KH, JG, P], F16)
                nc.sync.dma_start(wh_sb[:], wh[l].rearrange("d kt mt p q -> p d kt mt q"))
                nc.vector.memset(hist[:, :, :, 0, :], 0.0)

                for tb in range(0, T, U2):
                    # psum tile for U2 steps, both dirs
                    pt = sppool.tile([P, U2, 2, JG, BC], FP32, tag="pt")
                    # Per (u, d, mt) accumulation group: injector (identity
                    # matmul for xz, K=1 matmul for b1h) starts, then the Wh
                    # matmuls, kt=1 stops. The injector MUST be emitted
                    # immediately before its Wh matmuls: emitted early (it
                    # has no recurrent deps) the scheduler hoists it and the
                    # long-open interleaved groups corrupt the PSUM bank.
                    for u in range(U2):
                        t = tb + u
                        for d in range(2):
                            for mt in range(JG):
                                if mt < 4:
                                    nc.tensor.matmul(
                                        pt[:, u, d, mt, :],
                                        ident[:],
                                        xpz[:, d, t, mt, :],
                                        start=True,
                                        stop=False,
                                    )
                                else:
                                    nc.tensor.matmul(
                                        pt[:, u, d, mt, :],
                                        b1h_sb[0:1, l, d, mt - 4, :],
                                        ones16[:],
                                        start=True,
                                        stop=False,
                                    )
                                for kt in range(KH):
                                    nc.tensor.matmul(
                                        pt[:, u, d, mt, :],
                                        wh_sb[:, d, kt, mt, :],
                                        hist[:, d, kt, t, :],
                                        start=False,
                                        stop=(kt == KH - 1),
                                    )
                            # elementwise ops of a direction stay on one
                            # engine (d0 -> DVE, d1 -> GpSimd): each engine's
                            # in-order stream is a single dependency chain, so
                            # the two chains pipeline against each other. hm
                            # reads PSUM, which GpSimd cannot, so both hms run
                            # on DVE.
                            ew = nc.vector if d == 0 else nc.gpsimd
                            zr = gpool.tile([P, 4, BC], FP32, tag=f"zr{d}")
                            nc.scalar.activation(
                                zr[:], pt[:, u, d, 0:4, :], AF.Sigmoid
                            )
                            hm = gpool.tile([P, JH, BC], FP32, tag=f"hm{d}")
                            nc.vector.scalar_tensor_tensor(
                                hm[:], pt[:, u, d, 4:6, :], 0.0, zr[:, 2:4, :],
                                OP.add, OP.mult,
                            )
                            av = gpool.tile([P, JH, BC], FP32, tag=f"av{d}")
                            ew.tensor_tensor(
                                av[:], hm[:], xph[:, d, t, :, :], OP.add
                            )
                            hh = gpool.tile([P, JH, BC], FP32, tag=f"hh{d}")
                            nc.scalar.activation(hh[:], av[:], AF.Tanh)
                            dd = gpool.tile([P, JH, BC], FP32, tag=f"dd{d}")
                            ew.tensor_tensor(
                                dd[:], hist[:, d, :, t, :], hh[:], OP.subtract
                            )
                            ee = gpool.tile([P, JH, BC], FP32, tag=f"ee{d}")
                            ew.tensor_tensor(
                                ee[:], zr[:, 0:2, :], dd[:], OP.mult
                            )
                            ew.tensor_tensor(
                                hist[:, d, :, t + 1, :], hh[:], ee[:], OP.add
                            )

        # ---- head: leaks branch + folded BN/FC/BN/sigmoid
        with (
            tc.tile_pool(name="head", bufs=1) as hpool,
            tc.tile_pool(name="hpsum", bufs=2, space="PSUM") as hppool,
        ):
            lkw0 = hpool.tile([P, 2, P], F16)
            nc.sync.dma_start(lkw0[:], lw0[:])
            lkw1 = hpool.tile([20, 2, P], F16)
            nc.sync.dma_start(lkw1[:], lw1[:])
            lkb = hpool.tile([P, 2], FP32)
            nc.sync.dma_start(lkb[:], lb[:])
            lkx0 = hpool.tile([P, BCH], F16)
            nc.sync.dma_start(lkx0[:], leakst[0:P, :])
            lkx1 = hpool.tile([20, BCH], F16)
            nc.sync.dma_start(lkx1[:], leakst[P:148, :])

            lks = hpool.tile([P, 2, BCH], F16)
            for mt in range(2):
                lp = hppool.tile([P, BCH], FP32, tag="lp")
                nc.tensor.matmul(lp[:], lkw0[:, mt, :], lkx0[:], start=True, stop=False)
                nc.tensor.matmul(lp[:], lkw1[:, mt, :], lkx1[:], start=False, stop=True)
                nc.scalar.activation(
                    lks[:, mt, :], lp[:], AF.Relu, bias=lkb[:, mt : mt + 1]
                )

            w1_sb = hpool.tile([P, 10, 2, P], F16)
            nc.sync.dma_start(w1_sb[:], w1[:].rearrange("kt mt p q -> p kt mt q"))
            b1_sb = hpool.tile([P, 2], FP32)
            nc.sync.dma_start(b1_sb[:], b1p[:])
            wc_sb = hpool.tile([P, 2], F16)
            nc.sync.dma_start(wc_sb[:], wc[:])
            bc_sb = hpool.tile([1, 1], FP32)
            nc.sync.dma_start(bc_sb[:], bc_b[:])

            # concat order per original example-half: [fw(2 tiles), bw(2)]
            # for code (cols 0:4) then comment (cols 4:8), then leaks.
            fin16 = hpool.tile([P, 8, BCH], F16)
            for half in range(2):
                c0 = half * BCH
                for d in range(2):
                    for j in range(JH):
                        nc.vector.tensor_copy(
                            fin16[:, half * 4 + d * 2 + j, :],
                            hist[:, d, j, T, c0 : c0 + BCH],
                        )
            rhs_tiles = [fin16[:, k, :] for k in range(8)]
            rhs_tiles.append(lks[:, 0, :])
            rhs_tiles.append(lks[:, 1, :])

            yt = hpool.tile([P, 2, BCH], F16)
            for mt in range(2):
                hp = hppool.tile([P, BCH], FP32, tag="hp")
                for kt in range(10):
                    nc.tensor.matmul(
                        hp[:],
                        w1_sb[:, kt, mt, :],
                        rhs_tiles[kt],
                        start=(kt == 0),
                        stop=(kt == 9),
                    )
                nc.scalar.activation(
                    yt[:, mt, :], hp[:], AF.Relu, bias=b1_sb[:, mt : mt + 1]
                )

            op_ = hppool.tile([1, BCH], FP32, tag="op")
            for kt in range(2):
                nc.tensor.matmul(
                    op_[:],
                    wc_sb[:, kt : kt + 1],
                    yt[:, kt, :],
                    start=(kt == 0),
                    stop=(kt == 1),
                )
            res = hpool.tile([1, BCH], FP32)
            nc.scalar.activation(res[:], op_[:], AF.Sigmoid, bias=bc_sb[0:1, 0:1])
            nc.sync.dma_start(out[:], res[:])

    nc.compile()
    return nc


def prep_inputs(inputs, T=512, n_layers=NLAY):
    """Host-side: shard + pre-layout all tensors. Returns in_maps list."""
    ci = np.asarray(inputs["comment_indices"]).astype(np.int32)
    co = np.asarray(inputs["code_indices"]).astype(np.int32)
    emb_f = np.ascontiguousarray(
        np.asarray(inputs["embed_table"], np.float32)
    ).astype(NP_F16)
    gwx = np.asarray(inputs["gru_Wx"], np.float32)
    gwh = np.asarray(inputs["gru_Wh"], np.float32)
    gb = np.asarray(inputs["gru_b"], np.float32)

    wx_t = np.ascontiguousarray(
        gwx[:n_layers].reshape(n_layers, 2, JX, P, JG, P).transpose(0, 1, 2, 4, 3, 5)
    ).astype(NP_F16)
    wh_t = np.ascontiguousarray(
        gwh[:n_layers].reshape(n_layers, 2, KH, P, JG, P).transpose(0, 1, 2, 4, 3, 5)
    ).astype(NP_F16)

    pb = gb[:n_layers, :, 0, :].copy()  # [nl, 2, 768]
    pb[:, :, : 2 * H] += gb[:n_layers, :, 1, : 2 * H]
    pbias_h = np.ascontiguousarray(
        pb.reshape(n_layers, 2, JG, P).transpose(3, 0, 1, 2)
    ).astype(np.float32)
    b1h_h = np.ascontiguousarray(
        gb[:n_layers, :, 1, 2 * H :].reshape(1, n_layers, 2, JH, P)
    ).astype(NP_F16)

    s1 = np.asarray(inputs["bn1_gamma"], np.float32) / np.sqrt(
        np.asarray(inputs["bn1_var"], np.float32) + EPS
    )
    t1 = (
        np.asarray(inputs["bn1_beta"], np.float32)
        - np.asarray(inputs["bn1_mean"], np.float32) * s1
    )
    fc1 = np.asarray(inputs["fc1_W"], np.float32)
    w1p = fc1 * s1[:, None]
    b1v = t1 @ fc1 + np.asarray(inputs["fc1_b"], np.float32)
    s2 = np.asarray(inputs["bn2_gamma"], np.float32) / np.sqrt(
        np.asarray(inputs["bn2_var"], np.float32) + EPS
    )
    t2 = (
        np.asarray(inputs["bn2_beta"], np.float32)
        - np.asarray(inputs["bn2_mean"], np.float32) * s2
    )
    clsw = np.asarray(inputs["cls_W"], np.float32)
    wcp = clsw * s2[:, None]
    bcp = (t2 @ clsw + np.asarray(inputs["cls_b"], np.float32)).reshape(1, 1)

    w1_t = np.ascontiguousarray(w1p.reshape(10, P, 2, P).transpose(0, 2, 1, 3)).astype(
        NP_F16
    )
    b1p_h = np.ascontiguousarray(b1v.reshape(2, P).T).astype(np.float32)
    wc_h = np.ascontiguousarray(wcp.reshape(2, P).T).astype(NP_F16)

    lw = np.asarray(inputs["leaks_W"], np.float32)
    lw0_h = np.ascontiguousarray(lw[:P].reshape(P, 2, P)).astype(NP_F16)
    lw1_h = np.ascontiguousarray(lw[P:].reshape(20, 2, P)).astype(NP_F16)
    lb_h = np.ascontiguousarray(
        np.asarray(inputs["leaks_b"], np.float32).reshape(2, P).T
    ).astype(np.float32)
    leaks = np.asarray(inputs["leaks_indices"], np.float32)

    shared = dict(
        emb=emb_f, wx=wx_t, wh=wh_t, pbias=pbias_h, b1hs=b1h_h,
        w1=w1_t, b1p=b1p_h, wc=wc_h, bc=bcp.astype(np.float32),
        lw0=lw0_h, lw1=lw1_h, lb=lb_h,
    )
    in_maps = []
    for c in range(NCORES):
        exs = slice(BCH * c, BCH * c + BCH)
        merged = np.concatenate([co[exs, :T], ci[exs, :T]], 0)  # [8, T]
        idxw_h = np.ascontiguousarray(
            merged.reshape(BC, T // P, P).transpose(2, 0, 1).reshape(P, -1)
        ).astype(np.int32)
        lkt = np.ascontiguousarray(leaks[exs].T).astype(NP_F16)
        m = dict(shared)
        m["idxw"] = idxw_h
        m["leakst"] = lkt
        in_maps.append(m)
    return in_maps


def kernel(**inputs) -> np.ndarray:
    from concourse.bass_utils import run_bass_kernel_spmd

    nc = build_nc(T=512)
    in_maps = prep_inputs(inputs, T=512)
    res = run_bass_kernel_spmd(nc, in_maps, list(range(NCORES)))
    outs = [np.asarray(res.results[c]["out"]).reshape(-1) for c in range(NCORES)]
    return np.concatenate(outs).astype(np.float32)


if __name__ == "__main__":
    sys.path.insert(0, "/root/problem")
    import reference

    inp = {k: np.asarray(v) for k, v in reference.setup_inputs().items()}
    got = kernel(**inp)
    print("kernel out:", got[:8])


# revision 11
# speedup vs baseline: 1.5545x; 1.5545x over previous
"""Trainium2 Bass kernel for nn_BaselineModel_27298812133937.

Model: two [32,512] token sequences -> shared embedding [50000,512] ->
3 stacked bi-GRU layers (H=256, Keras reset_after) -> last states,
plus a leaks MLP branch, then BN/FC/BN/sigmoid head -> [32].

Sharding: the two sequences share GRU weights, so they merge into a
batch of 64. Each of the 8 cores takes 8 merged examples (4 code + 4
comment of the same original examples), runs the full network for its
shard with no cross-core communication, and computes the head for its
4 original examples. Host concatenates the 8x[4] outputs.

v2 design (everything SBUF-resident after the embedding):
- float16 storage throughout (same matmul/DVE speed as bf16, 8x better
  mantissa).
- Input projections xp = Wx^T x + b are computed per layer into
  SBUF-resident xpz (z,r gates, f16) / xph (h gate, f32) buffers; the
  projection's moving operand for layers 1,2 is the previous layer's
  state history read directly from SBUF (no DRAM round trip).
- The scan keeps the full state history of both directions in a single
  SBUF tile hist[d, jh, t+1, b]; step t reads slot t and writes slot
  t+1. The history IS the next layer's input and the head's input.
- Per step, the z/r input projections and the recurrent bias b1h are
  injected into PSUM by identity / K=1 matmuls (same-region
  accumulation groups), so the gate math is only: sigmoid (Act),
  hm/av/stn (DVE), dd/ee (GpSimd), tanh (Act). The two directions stay
  in separate instructions: they are independent dependency chains
  that hide each other's latency.
"""

import os
import sys

import numpy as np

for _p in ("/opt/trn_rl_repo",):
    if os.path.isdir(_p) and _p not in sys.path:
        sys.path.insert(0, _p)

import concourse.bass as bass
import concourse.tile as tile
from concourse import bacc, mybir
from concourse.masks import make_identity

import ml_dtypes

FP32 = mybir.dt.float32
F16 = mybir.dt.float16
I32 = mybir.dt.int32
AF = mybir.ActivationFunctionType
OP = mybir.AluOpType
NP_F16 = np.float16

V, E, H, NLAY = 50000, 512, 256, 3
EPS = 1e-3
P = 128
JX = E // P        # 4  x-feature tiles
JG = 3 * H // P    # 6  gate tiles
JH = H // P        # 2  hidden tiles
KH = H // P        # 2  Wh contraction tiles
BC = 8             # merged examples per core
BCH = 4            # head (original) examples per core
NCORES = 8
U2 = 2             # psum tile groups 2 time steps


def build_nc(T=512, n_layers=NLAY, debug=False):
    assert T % P == 0 and T % U2 == 0
    NCH = BC * (T // P)

    nc = bacc.Bacc("TRN2", target_bir_lowering=False, debug=debug)

    def din(name, shape, dt):
        return nc.declare_dram_parameter(name, list(shape), dt, False)

    emb = din("emb", [V, E], F16)
    idxw = din("idxw", [P, NCH], I32)
    wx = din("wx", [n_layers, 2, JX, JG, P, P], F16)
    wh = din("wh", [n_layers, 2, KH, JG, P, P], F16)
    pbias = din("pbias", [P, n_layers, 2, JG], FP32)
    b1hs = din("b1hs", [P, n_layers, 2, JH, P], F16)  # row-0-only stationary
    w1 = din("w1", [10, 2, P, P], F16)
    b1p = din("b1p", [P, 2], FP32)
    wc = din("wc", [P, 2], F16)
    bc_b = din("bc", [1, 1], FP32)
    lw0 = din("lw0", [P, 2, P], F16)
    lw1 = din("lw1", [20, 2, P], F16)
    lb = din("lb", [P, 2], FP32)
    leakst = din("leakst", [148, BCH], F16)

    out = nc.declare_dram_parameter("out", [1, BCH], FP32, True)

    # embedding output (layer-0 input), transposed layout
    x0 = nc.dram_tensor("x0", [P, JX, T, BC], F16)

    with tile.TileContext(nc) as tc, tc.tile_pool(name="const", bufs=1) as cpool:
        # ---- persistent SBUF state
        ident = cpool.tile([P, P], F16)
        make_identity(nc, ident[:])
        idx_sb = cpool.tile([P, NCH], I32)
        nc.sync.dma_start(idx_sb[:], idxw[:])
        pb_sb = cpool.tile([P, n_layers, 2, JG], FP32)
        nc.sync.dma_start(pb_sb[:], pbias[:])
        b1h_sb = cpool.tile([P, n_layers, 2, JH, P], F16)
        nc.sync.dma_start(b1h_sb[:], b1hs[:])
        # e0: ones in partition 0, zeros elsewhere; the b1h injection is a
        # K=128 outer-product matmul so the PE stream stays shape-uniform
        # (mixed stationary shapes stall the LDWEIGHTS/MATMUL pipeline).
        e0_16 = cpool.tile([P, BC], F16)
        nc.vector.memset(e0_16[:], 0.0)
        nc.vector.memset(e0_16[0:1, :], 1.0)

        # xp buffers: bw (d=1) xp is stored in scan order (time-reversed).
        xpz = cpool.tile([P, 2, T, 4, BC], F16)
        xph = cpool.tile([P, 2, T, JH, BC], F16)
        # state history, both dirs; slot 0 is h=0, slot t+1 = state after
        # step t (bw steps count from the end of the sequence). A single
        # buffer: each layer's projection reads the whole history before
        # that layer's scan starts overwriting it.
        hist = cpool.tile([P, 2, JH, T + 1, BC], F16)

        # ---- phase 1: embedding gather -> x0 (layer-0 input, transposed)
        with (
            tc.tile_pool(name="erow", bufs=3) as epool,
            tc.tile_pool(name="estage", bufs=2) as espool,
            tc.tile_pool(name="epsum", bufs=4, space="PSUM") as eppool,
        ):
            for tc_i in range(T // P):
                stages = [
                    espool.tile([P, P, BC], F16, name=f"estg{j}", tag=f"st{j}")
                    for j in range(JX)
                ]
                for bi in range(BC):
                    ch = bi * (T // P) + tc_i
                    g = epool.tile([P, E], F16)
                    nc.gpsimd.indirect_dma_start(
                        out=g[:],
                        out_offset=None,
                        in_=emb[:],
                        in_offset=bass.IndirectOffsetOnAxis(
                            ap=idx_sb[:, ch : ch + 1], axis=0
                        ),
                    )
                    for j in range(JX):
                        pst = eppool.tile([P, P], F16)
                        nc.tensor.transpose(pst[:], g[:, j * P : (j + 1) * P], ident[:])
                        if (bi + j) % 2 == 0:
                            nc.vector.tensor_copy(stages[j][:, :, bi], pst[:])
                        else:
                            nc.scalar.copy(stages[j][:, :, bi], pst[:])
                for j in range(JX):
                    nc.sync.dma_start(
                        x0[:, j, tc_i * P : (tc_i + 1) * P, :], stages[j][:]
                    )

        # ---- per-layer: projection into SBUF xp, then scan into hist
        for l in range(n_layers):
            # -- input projection: xp = Wx^T x (+bias) -> SBUF
            with (
                tc.tile_pool(name="wts", bufs=1) as wpool,
                tc.tile_pool(name="ppsum", bufs=3, space="PSUM") as pppool,
                tc.tile_pool(name="xchunk", bufs=2) as xcpool,
            ):
                wx_sb = wpool.tile([P, 2, JX, JG, P], F16)
                nc.sync.dma_start(wx_sb[:], wx[l].rearrange("d kt mt p q -> p d kt mt q"))

                CT = 64  # chunk of time steps
                NCK = T // CT
                for c in range(NCK):
                    t0, t1 = c * CT, (c + 1) * CT
                    if l == 0:
                        xch = xcpool.tile([P, JX, CT, BC], F16, tag="xch")
                        nc.sync.dma_start(xch[:], x0[:, :, t0:t1, :])
                        movers = [xch[:, kt, :, :] for kt in range(JX)]
                    else:
                        # fw states for t in [t0,t1): slots t+1
                        # bw states for t: slots T-t (descending)
                        movers = [
                            hist[:, 0, 0, t0 + 1 : t1 + 1, :],
                            hist[:, 0, 1, t0 + 1 : t1 + 1, :],
                            hist[:, 1, 0, T - t0 : T - t1 : -1, :],
                            hist[:, 1, 1, T - t0 : T - t1 : -1, :],
                        ]
                    for d in range(2):
                        for mt in range(JG):
                            ps = pppool.tile([P, CT, BC], FP32)
                            for kt in range(JX):
                                nc.tensor.matmul(
                                    ps[:],
                                    wx_sb[:, d, kt, mt, :],
                                    movers[kt],
                                    start=(kt == 0),
                                    stop=(kt == JX - 1),
                                )
                            # fw consumes time ascending; bw scan step u
                            # consumes t=T-1-u, so store bw reversed.
                            if d == 0:
                                tdst = slice(t0, t1)
                            else:
                                tdst = slice(T - 1 - t0, T - 1 - t1 if t1 < T else None, -1)
                            if mt < 4:
                                nc.vector.tensor_scalar_add(
                                    xpz[:, d, tdst, mt, :], ps[:],
                                    pb_sb[:, l, d, mt : mt + 1],
                                )
                            else:
                                nc.scalar.activation(
                                    xph[:, d, tdst, mt - 4, :], ps[:],
                                    AF.Identity,
                                    bias=pb_sb[:, l, d, mt : mt + 1],
                                )

            # -- scan
            with (
                tc.tile_pool(name="gates", bufs=8) as gpool,
                tc.tile_pool(name="spsum", bufs=4, space="PSUM") as sppool,
                tc.tile_pool(name="wts2", bufs=1) as wpool2,
            ):
                wh_sb = wpool2.tile([P, 2, KH, JG, P], F16)
                nc.sync.dma_start(wh_sb[:], wh[l].rearrange("d kt mt p q -> p d kt mt q"))
                nc.vector.memset(hist[:, :, :, 0, :], 0.0)

                for tb in range(0, T, U2):
                    # psum tile for U2 steps, both dirs
                    pt = sppool.tile([P, U2, 2, JG, BC], FP32, tag="pt")
                    # Per (u, d, mt) accumulation group: injector (identity
                    # matmul for xz, K=1 matmul for b1h) starts, then the Wh
                    # matmuls, kt=1 stops. The injector MUST be emitted
                    # immediately before its Wh matmuls: emitted early (it
                    # has no recurrent deps) the scheduler hoists it and the
                    # long-open interleaved groups corrupt the PSUM bank.
                    for u in range(U2):
                        t = tb + u
                        for d in range(2):
                            for mt in range(JG):
                                if mt < 4:
                                    nc.tensor.matmul(
                                        pt[:, u, d, mt, :],
                                        ident[:],
                                        xpz[:, d, t, mt, :],
                                        start=True,
                                        stop=False,
                                    )
                                else:
                                    nc.tensor.matmul(
                                        pt[:, u, d, mt, :],
                                        b1h_sb[:, l, d, mt - 4, :],
                                        e0_16[:],
                                        start=True,
                                        stop=False,
                                    )
                                for kt in range(KH):
                                    nc.tensor.matmul(
                                        pt[:, u, d, mt, :],
                                        wh_sb[:, d, kt, mt, :],
                                        hist[:, d, kt, t, :],
                                        start=False,
                                        stop=(kt == KH - 1),
                                    )
                            ew = nc.vector
                            zr = gpool.tile([P, 4, BC], FP32, tag=f"zr{d}")
                            nc.scalar.activation(
                                zr[:], pt[:, u, d, 0:4, :], AF.Sigmoid
                            )
                            hm = gpool.tile([P, JH, BC], FP32, tag=f"hm{d}")
                            nc.vector.scalar_tensor_tensor(
                                hm[:], pt[:, u, d, 4:6, :], 0.0, zr[:, 2:4, :],
                                OP.add, OP.mult,
                            )
                            av = gpool.tile([P, JH, BC], FP32, tag=f"av{d}")
                            ew.tensor_tensor(
                                av[:], hm[:], xph[:, d, t, :, :], OP.add
                            )
                            hh = gpool.tile([P, JH, BC], FP32, tag=f"hh{d}")
                            nc.scalar.activation(hh[:], av[:], AF.Tanh)
                            dd = gpool.tile([P, JH, BC], FP32, tag=f"dd{d}")
                            nc.gpsimd.tensor_tensor(
                                dd[:], hist[:, d, :, t, :], hh[:], OP.subtract
                            )
                            ee = gpool.tile([P, JH, BC], FP32, tag=f"ee{d}")
                            nc.gpsimd.tensor_tensor(
                                ee[:], zr[:, 0:2, :], dd[:], OP.mult
                            )
                            ew.tensor_tensor(
                                hist[:, d, :, t + 1, :], hh[:], ee[:], OP.add
                            )

        # ---- head: leaks branch + folded BN/FC/BN/sigmoid
        with (
            tc.tile_pool(name="head", bufs=1) as hpool,
            tc.tile_pool(name="hpsum", bufs=2, space="PSUM") as hppool,
        ):
            lkw0 = hpool.tile([P, 2, P], F16)
            nc.sync.dma_start(lkw0[:], lw0[:])
            lkw1 = hpool.tile([20, 2, P], F16)
            nc.sync.dma_start(lkw1[:], lw1[:])
            lkb = hpool.tile([P, 2], FP32)
            nc.sync.dma_start(lkb[:], lb[:])
            lkx0 = hpool.tile([P, BCH], F16)
            nc.sync.dma_start(lkx0[:], leakst[0:P, :])
            lkx1 = hpool.tile([20, BCH], F16)
            nc.sync.dma_start(lkx1[:], leakst[P:148, :])

            lks = hpool.tile([P, 2, BCH], F16)
            for mt in range(2):
                lp = hppool.tile([P, BCH], FP32, tag="lp")
                nc.tensor.matmul(lp[:], lkw0[:, mt, :], lkx0[:], start=True, stop=False)
                nc.tensor.matmul(lp[:], lkw1[:, mt, :], lkx1[:], start=False, stop=True)
                nc.scalar.activation(
                    lks[:, mt, :], lp[:], AF.Relu, bias=lkb[:, mt : mt + 1]
                )

            w1_sb = hpool.tile([P, 10, 2, P], F16)
            nc.sync.dma_start(w1_sb[:], w1[:].rearrange("kt mt p q -> p kt mt q"))
            b1_sb = hpool.tile([P, 2], FP32)
            nc.sync.dma_start(b1_sb[:], b1p[:])
            wc_sb = hpool.tile([P, 2], F16)
            nc.sync.dma_start(wc_sb[:], wc[:])
            bc_sb = hpool.tile([1, 1], FP32)
            nc.sync.dma_start(bc_sb[:], bc_b[:])

            # concat order per original example-half: [fw(2 tiles), bw(2)]
            # for code (cols 0:4) then comment (cols 4:8), then leaks.
            fin16 = hpool.tile([P, 8, BCH], F16)
            for half in range(2):
                c0 = half * BCH
                for d in range(2):
                    for j in range(JH):
                        nc.vector.tensor_copy(
                            fin16[:, half * 4 + d * 2 + j, :],
                            hist[:, d, j, T, c0 : c0 + BCH],
                        )
            rhs_tiles = [fin16[:, k, :] for k in range(8)]
            rhs_tiles.append(lks[:, 0, :])
            rhs_tiles.append(lks[:, 1, :])

            yt = hpool.tile([P, 2, BCH], F16)
            for mt in range(2):
                hp = hppool.tile([P, BCH], FP32, tag="hp")
                for kt in range(10):
                    nc.tensor.matmul(
                        hp[:],
                        w1_sb[:, kt, mt, :],
                        rhs_tiles[kt],
                        start=(kt == 0),
                        stop=(kt == 9),
                    )
                nc.scalar.activation(
                    yt[:, mt, :], hp[:], AF.Relu, bias=b1_sb[:, mt : mt + 1]
                )

            op_ = hppool.tile([1, BCH], FP32, tag="op")
            for kt in range(2):
                nc.tensor.matmul(
                    op_[:],
                    wc_sb[:, kt : kt + 1],
                    yt[:, kt, :],
                    start=(kt == 0),
                    stop=(kt == 1),
                )
            res = hpool.tile([1, BCH], FP32)
            nc.scalar.activation(res[:], op_[:], AF.Sigmoid, bias=bc_sb[0:1, 0:1])
            nc.sync.dma_start(out[:], res[:])

    nc.compile()
    return nc


def prep_inputs(inputs, T=512, n_layers=NLAY):
    """Host-side: shard + pre-layout all tensors. Returns in_maps list."""
    ci = np.asarray(inputs["comment_indices"]).astype(np.int32)
    co = np.asarray(inputs["code_indices"]).astype(np.int32)
    emb_f = np.ascontiguousarray(
        np.asarray(inputs["embed_table"], np.float32)
    ).astype(NP_F16)
    gwx = np.asarray(inputs["gru_Wx"], np.float32)
    gwh = np.asarray(inputs["gru_Wh"], np.float32)
    gb = np.asarray(inputs["gru_b"], np.float32)

    wx_t = np.ascontiguousarray(
        gwx[:n_layers].reshape(n_layers, 2, JX, P, JG, P).transpose(0, 1, 2, 4, 3, 5)
    ).astype(NP_F16)
    wh_t = np.ascontiguousarray(
        gwh[:n_layers].reshape(n_layers, 2, KH, P, JG, P).transpose(0, 1, 2, 4, 3, 5)
    ).astype(NP_F16)

    pb = gb[:n_layers, :, 0, :].copy()  # [nl, 2, 768]
    pb[:, :, : 2 * H] += gb[:n_layers, :, 1, : 2 * H]
    pbias_h = np.ascontiguousarray(
        pb.reshape(n_layers, 2, JG, P).transpose(3, 0, 1, 2)
    ).astype(np.float32)
    b1h_h = np.zeros((P, n_layers, 2, JH, P), NP_F16)
    b1h_h[0] = gb[:n_layers, :, 1, 2 * H :].reshape(n_layers, 2, JH, P).astype(NP_F16)

    s1 = np.asarray(inputs["bn1_gamma"], np.float32) / np.sqrt(
        np.asarray(inputs["bn1_var"], np.float32) + EPS
    )
    t1 = (
        np.asarray(inputs["bn1_beta"], np.float32)
        - np.asarray(inputs["bn1_mean"], np.float32) * s1
    )
    fc1 = np.asarray(inputs["fc1_W"], np.float32)
    w1p = fc1 * s1[:, None]
    b1v = t1 @ fc1 + np.asarray(inputs["fc1_b"], np.float32)
    s2 = np.asarray(inputs["bn2_gamma"], np.float32) / np.sqrt(
        np.asarray(inputs["bn2_var"], np.float32) + EPS
    )
    t2 = (
        np.asarray(inputs["bn2_beta"], np.float32)
        - np.asarray(inputs["bn2_mean"], np.float32) * s2
    )
    clsw = np.asarray(inputs["cls_W"], np.float32)
    wcp = clsw * s2[:, None]
    bcp = (t2 @ clsw + np.asarray(inputs["cls_b"], np.float32)).reshape(1, 1)

    w1_t = np.ascontiguousarray(w1p.reshape(10, P, 2, P).transpose(0, 2, 1, 3)).astype(
        NP_F16
    )
    b1p_h = np.ascontiguousarray(b1v.reshape(2, P).T).astype(np.float32)
    wc_h = np.ascontiguousarray(wcp.reshape(2, P).T).astype(NP_F16)

    lw = np.asarray(inputs["leaks_W"], np.float32)
    lw0_h = np.ascontiguousarray(lw[:P].reshape(P, 2, P)).astype(NP_F16)
    lw1_h = np.ascontiguousarray(lw[P:].reshape(20, 2, P)).astype(NP_F16)
    lb_h = np.ascontiguousarray(
        np.asarray(inputs["leaks_b"], np.float32).reshape(2, P).T
    ).astype(np.float32)
    leaks = np.asarray(inputs["leaks_indices"], np.float32)

    shared = dict(
        emb=emb_f, wx=wx_t, wh=wh_t, pbias=pbias_h, b1hs=b1h_h,
        w1=w1_t, b1p=b1p_h, wc=wc_h, bc=bcp.astype(np.float32),
        lw0=lw0_h, lw1=lw1_h, lb=lb_h,
    )
    in_maps = []
    for c in range(NCORES):
        exs = slice(BCH * c, BCH * c + BCH)
        merged = np.concatenate([co[exs, :T], ci[exs, :T]], 0)  # [8, T]
        idxw_h = np.ascontiguousarray(
            merged.reshape(BC, T // P, P).transpose(2, 0, 1).reshape(P, -1)
        ).astype(np.int32)
        lkt = np.ascontiguousarray(leaks[exs].T).astype(NP_F16)
        m = dict(shared)
        m["idxw"] = idxw_h
        m["leakst"] = lkt
        in_maps.append(m)
    return in_maps


def kernel(**inputs) -> np.ndarray:
    from concourse.bass_utils import run_bass_kernel_spmd

    nc = build_nc(T=512)
    in_maps = prep_inputs(inputs, T=512)
    res = run_bass_kernel_spmd(nc, in_maps, list(range(NCORES)))
    outs = [np.asarray(res.results[c]["out"]).reshape(-1) for c in range(NCORES)]
    return np.concatenate(outs).astype(np.float32)


if __name__ == "__main__":
    sys.path.insert(0, "/root/problem")
    import reference

    inp = {k: np.asarray(v) for k, v in reference.setup_inputs().items()}
    got = kernel(**inp)
    print("kernel out:", got[:8])
